# revision 22
# baseline (speedup 1.0000x reference)
"""Trainium2 Bass kernel for nn_AttnLayer (dense_transformer, sum-normalized attention).

Reference computation (per batch b, all fp32):
    d      = X @ W1.T + T @ W2.T + (b1+b2)      X=in_seq, T=prev_target_seq
    S      = d @ E.T                            E=enc_seq
    ssum_l = sum_m S[l,m]                       (sum-normalization, NOT softmax)
    out    = (S @ E / ssum[:,None]) @ W3.T + b3

Algorithm (PE computes out = lhsT.T @ rhs, contraction over the partition dim).
The attention is linear, so S is never materialized:
    G  = E.T @ E      [e,e']  bf16 inputs, fp32 psum     (Gram, halves score FLOPs)
    H  = G @ W3T      [e,o]   bf16 x bf16
    dT = W1T.T @ X^T + W2T.T @ T^T + bd   [e,l]  f32r (fp32 bits, fast PE mode)
    O  = dT.T @ H     [l,o]   f32r, then * 1/ssum + b3

ssum numerics: catastrophic cancellation (min |ssum| ~ 0.05 vs typical ~700)
forces the denominator path to exact fp32:
    ssum = X @ v1 + T @ v2 + c0,  v1 = W1.T @ esum,  esum = sum_m E[m,:]
v1/v2/c0 are host-precomputed in fp64 (tiny O(LBD+D^2) input/weight reductions);
on device ssum is 8 fp32 PE matmuls per l-block with lhsT = xT block and
rhs = v1 column, accumulating in a PSUM column -- exact fp32 and the result
lands as a [128,1] per-partition column, directly usable as the output scale.
xT/tT are loaded once as fp32 and bitcast to f32r for the big matmuls (f32r is
fp32 bits; at moving-dim 512 it runs 1 cycle/row, full PE rate), so no tensor
is ever loaded twice. The numerator's bf16/f32r errors stay relative to the
numerator and cancel against the same 1/ssum.

Sharding: data-parallel over batch B=16 across 8 cores (2 batches per core).
All input loads on the Act HWDGE queue in dependency order (Gram inputs first,
then the d-chain t-pass, then x-pass); output stores on the idle Pool SWDGE.
PE is the bottleneck and is kept continuously busy (full 2.4 GHz p-state).
"""

import os

os.environ.setdefault("MYCRO_LOCAL_CACHE", "1")

import numpy as np
import ml_dtypes

import concourse.bass as bass
from concourse import bacc
import concourse.mybir as mybir
import concourse.tile as tile
from concourse.bass_utils import run_bass_kernel_spmd

# Problem shape (hardcoded per contract)
L = 1024      # L_in == L_enc
B = 16
D = 512       # D_in == D_enc == D_emb
N_CORES = 8
BPC = B // N_CORES   # batches per core
P = 128
NE = D // P          # 4 chunks of the 512-wide contraction axes
NM = L // P          # 8 chunks of the L_enc axis
NL = L // P          # 8 chunks of the L_in axis
NLH = 2              # l processed in halves of 512 (moving-operand max for 4-byte)
LH = L // NLH

F32 = mybir.dt.float32
F32R = mybir.dt.float32r
BF16 = mybir.dt.bfloat16
BF16_NP = np.dtype(ml_dtypes.bfloat16)

# vecs packing (columns of the [P, 22] fp32 host-packed vector block):
#   v1 for b: cols [b*NE, b*NE+NE)          (X matvec weights, i on partitions)
#   v2 for b: cols [8 + b*NE, ...)
#   bd:       cols [16, 20)                 (b1+b2, e on partitions)
#   c0 for b: col 20+b                      (esum . bd, replicated)
V1C = 0
V2C = 2 * NE
BDC = 4 * NE
C0C = 4 * NE + NE


def build_nc():
    nc = bacc.Bacc(None, target_bir_lowering=False, debug=False)

    xT_d = nc.declare_dram_parameter("xT", [BPC, D, L], F32, isOutput=False)
    tT_d = nc.declare_dram_parameter("tT", [BPC, D, L], F32, isOutput=False)
    enb_d = nc.declare_dram_parameter("enb", [BPC, L, D], BF16, isOutput=False)
    w1t_d = nc.declare_dram_parameter("w1t", [D, D], F32R, isOutput=False)   # [i, e]
    w2t_d = nc.declare_dram_parameter("w2t", [D, D], F32R, isOutput=False)   # [j, e]
    w3tb_d = nc.declare_dram_parameter("w3tb", [D, D], BF16, isOutput=False)  # [e, o]
    vecs_d = nc.declare_dram_parameter("vecs", [P, 22], F32, isOutput=False)
    b3bc_d = nc.declare_dram_parameter("b3bc", [P, D], F32, isOutput=False)
    out_d = nc.declare_dram_parameter("out", [BPC, L, D], F32, isOutput=True)

    AF = mybir.ActivationFunctionType
    ALU = mybir.AluOpType

    with tile.TileContext(nc) as tc:
        with (
            tc.tile_pool(name="wpool", bufs=1) as wpool,
            tc.tile_pool(name="big", bufs=1) as big,
            tc.tile_pool(name="opool", bufs=4) as opool,
            tc.tile_pool(name="ps", bufs=1, space="PSUM") as ps,
        ):
            w1t = wpool.tile([P, NE, D], F32R, name="w1t")
            w2t = wpool.tile([P, NE, D], F32R, name="w2t")
            w3tb = wpool.tile([P, NE, D], BF16, name="w3tb")
            vecs = wpool.tile([P, 22], F32, name="vecs")
            b3bc = wpool.tile([P, D], F32, name="b3bc")

            # PE p-state warm-up: dummy bf16 matmuls keep PE busy while the
            # first loads land, so the real stream dispatches at full clock.
            # (memzero on DVE: Pool's library preamble would delay it ~1us)
            dzero = wpool.tile([P, D], BF16, name="dzero")
            nc.vector.memzero(dzero)
            # Act warm-up: the first activation triggers a ~1.3us activation
            # table load; trigger it now so the G copies aren't delayed
            actwarm = wpool.tile([P, 1], F32, name="actwarm")
            nc.scalar.activation(actwarm, dzero[:, :1], AF.Copy)
            for _ in range(6):
                warm_ps = ps.tile([P, D], F32, name="warm_ps", tag="ps8", bufs=8)
                nc.tensor.matmul(warm_ps, dzero[:, :P], dzero, start=True, stop=True)

            # ---- all big loads up front on the SP HWDGE queue, in
            # consumption order (so stores, which follow on the same queue,
            # never delay the second batch's loads); weights/vecs on the
            # Pool SWDGE queue in their own need-by order ----
            enbs, xTs, tTs = [], [], []
            for b in range(BPC):
                enb = big.tile([P, NM, D], BF16, name="enb", tag="enb", bufs=2)
                xT = big.tile([P, NE, L], F32, name="xT", tag="xT", bufs=2)
                tT = big.tile([P, NE, L], F32, name="tT", tag="tT", bufs=2)
                enbs.append(enb); xTs.append(xT); tTs.append(tT)
                for mp in range(0, NM, 2):
                    nc.sync.dma_start(
                        out=enb[:, mp : mp + 2, :],
                        in_=enb_d[b, mp * P : (mp + 2) * P, :]
                        .rearrange("(c p) e -> p c e", p=P))
                for k in range(NE):
                    nc.sync.dma_start(
                        out=tT[:, k, :], in_=tT_d[b, k * P : (k + 1) * P, :])
                for k in range(NE):
                    nc.sync.dma_start(
                        out=xT[:, k, :], in_=xT_d[b, k * P : (k + 1) * P, :])
            nc.gpsimd.dma_start(
                out=w3tb, in_=w3tb_d.rearrange("(c p) e -> p c e", p=P))
            for k in range(NE):
                nc.gpsimd.dma_start(
                    out=w2t[:, k, :], in_=w2t_d[k * P : (k + 1) * P, :])
            for k in range(NE):
                nc.gpsimd.dma_start(
                    out=w1t[:, k, :], in_=w1t_d[k * P : (k + 1) * P, :])
            nc.gpsimd.dma_start(out=vecs, in_=vecs_d[:, :])
            nc.gpsimd.dma_start(out=b3bc, in_=b3bc_d[:, :])

            for b in range(BPC):
                enb, xT, tT = enbs[b], xTs[b], tTs[b]

                # f32r rounding copies of tT, emitted first so Act runs them
                # while PE is still on the Gram (before the G copies in Act's
                # queue); bufs=4 keeps all chunks live until the d-pass reads
                # them, so the ring never blocks the queue
                tTr = [big.tile([P, L], F32R, name="tTr", tag="tTr", bufs=4)
                       for _ in range(NE)]
                for k in range(NE):
                    nc.vector.tensor_copy(tTr[k], tT[:, k, :])

                # ---- G[e,e'] = E.T @ E (bf16 Gram), mc-outer so each
                # arriving enb chunk feeds all 4 psum tiles ----
                G_sb = big.tile([P, NE, D], BF16, name="G_sb", tag="G", bufs=2)
                g_ps = [ps.tile([P, D], F32, name="g_ps", tag="ps8", bufs=8)
                        for _ in range(NE)]
                for mc in range(NM - 1):
                    for gc in range(NE):
                        nc.tensor.matmul(
                            g_ps[gc],
                            enb[:, mc, gc * P : (gc + 1) * P],
                            enb[:, mc, :],
                            start=(mc == 0), stop=False,
                        )
                # staggered finish: each gc's copy overlaps the others' tails
                for gc in range(NE):
                    nc.tensor.matmul(
                        g_ps[gc],
                        enb[:, NM - 1, gc * P : (gc + 1) * P],
                        enb[:, NM - 1, :],
                        start=False, stop=True,
                    )
                    nc.scalar.activation(G_sb[:, gc, :], g_ps[gc], AF.Copy)

                # ---- H[e,o] = G @ W3T (bf16) ----
                H_sb = big.tile([P, NE, D], F32R, name="H_sb", tag="H", bufs=2)
                for hc in range(NE):
                    h_ps = ps.tile([P, D], F32, name="h_ps", tag="ps8", bufs=8)
                    for kc in range(NE):
                        nc.tensor.matmul(
                            h_ps,
                            G_sb[:, kc, hc * P : (hc + 1) * P],
                            w3tb[:, kc, :],
                            start=(kc == 0), stop=(kc == NE - 1),
                        )
                    nc.scalar.activation(H_sb[:, hc, :], h_ps, AF.Copy)

                # ---- dT[e,l] = W1T.T @ X^T + W2T.T @ T^T + bd (f32r) ----
                # k-outer so each arriving tT/xT chunk feeds all 8 psum tiles;
                # t-pass first (tT chunks load before xT chunks).
                dT = big.tile([P, NE, L], F32R, name="dT", tag="dT", bufs=2)
                d_ps = [[ps.tile([P, LH], F32, name="d_ps", tag="ps8", bufs=8)
                         for _ in range(NE)] for _ in range(NLH)]
                # t-pass (tTr already rounded), then x-pass with inline
                # rounding copies; the final k chunk is emitted per-tile with
                # its +bd copy-out so the DVE drain staggers with the PE tail
                for k in range(NE):
                    for lh in range(NLH):
                        for ec in range(NE):
                            nc.tensor.matmul(
                                d_ps[lh][ec],
                                w2t[:, k, ec * P : (ec + 1) * P],
                                tTr[k][:, lh * LH : (lh + 1) * LH],
                                start=(k == 0), stop=False,
                            )
                xTr_ring = []
                for k in range(NE):
                    xTr = big.tile([P, L], F32R, name="xTr", tag="xTr", bufs=2)
                    nc.scalar.activation(xTr, xT[:, k, :], AF.Copy)
                    xTr_ring.append(xTr)
                    if k == NE - 1:
                        break
                    for lh in range(NLH):
                        for ec in range(NE):
                            nc.tensor.matmul(
                                d_ps[lh][ec],
                                w1t[:, k, ec * P : (ec + 1) * P],
                                xTr[:, lh * LH : (lh + 1) * LH],
                                start=False, stop=False,
                            )
                for lh in range(NLH):
                    for ec in range(NE):
                        nc.tensor.matmul(
                            d_ps[lh][ec],
                            w1t[:, NE - 1, ec * P : (ec + 1) * P],
                            xTr_ring[NE - 1][:, lh * LH : (lh + 1) * LH],
                            start=False, stop=True,
                        )
                        nc.vector.tensor_scalar_add(
                            dT[:, ec, lh * LH : (lh + 1) * LH], d_ps[lh][ec],
                            vecs[:, BDC + ec : BDC + ec + 1])

                # ---- ssum[l] = X@v1 + T@v2 + c0, exact fp32 PE matvecs into
                # PSUM columns; rcols = 1/(ssum+c0) per l-block on Act ----
                # two s tiles of 4 columns each so each PSUM slot's rcols
                # reads finish well before the O ring wraps onto it
                s_ps = [ps.tile([P, D], F32, name="s_ps", tag="ps8", bufs=8)
                        for _ in range(2)]
                rcols = big.tile([P, NL], F32, name="rcols", tag="rcols", bufs=2)
                for lc in range(NL):
                    cols = slice(lc, lc + 1)
                    scols = slice(lc % 4, lc % 4 + 1)
                    sp = s_ps[lc // 4]
                    i = 0
                    for src, vc0 in ((tT, V2C + b * NE), (xT, V1C + b * NE)):
                        for k in range(NE):
                            nc.tensor.matmul(
                                sp[:, scols],
                                src[:, k, lc * P : (lc + 1) * P],
                                vecs[:, vc0 + k : vc0 + k + 1],
                                start=(i == 0), stop=(i == 2 * NE - 1),
                            )
                            i += 1
                    nc.vector.tensor_scalar_add(
                        rcols[:, cols], sp[:, scols],
                        vecs[:, C0C + b : C0C + b + 1])
                    nc.vector.reciprocal(rcols[:, cols], rcols[:, cols])

                # ---- O[l,o] = dT.T @ H, * 1/ssum, + b3 ----
                for lc in range(NL):
                    o_ps = ps.tile([P, D], F32, name="o_ps", tag="ps8", bufs=8)
                    for ec in range(NE):
                        nc.tensor.matmul(
                            o_ps,
                            dT[:, ec, lc * P : (lc + 1) * P],
                            H_sb[:, ec, :],
                            start=(ec == 0), stop=(ec == NE - 1),
                        )
                    o_sb = opool.tile([P, D], F32, name="o_sb")
                    nc.vector.scalar_tensor_tensor(
                        o_sb, o_ps, rcols[:, lc : lc + 1], b3bc,
                        op0=ALU.mult, op1=ALU.add)
                    nc.sync.dma_start(
                        out=out_d[b, lc * P : (lc + 1) * P, :], in_=o_sb)

    nc.finalize()
    return nc


def _make_in_maps(in_seq, enc_seq, prev_target_seq, W_in2enc, b_in2enc,
                  W_lab2enc, b_lab2enc, W_enc2in, b_enc2in):
    f32 = np.float32
    f64 = np.float64
    W1 = np.asarray(W_in2enc, f32)
    W2 = np.asarray(W_lab2enc, f32)
    W3 = np.asarray(W_enc2in, f32)
    bd = (np.asarray(b_in2enc, f64) + np.asarray(b_lab2enc, f64)).astype(f32)
    w1t = np.ascontiguousarray(W1.T)   # [i, e]
    w2t = np.ascontiguousarray(W2.T)   # [j, e]
    w3tb = np.ascontiguousarray(W3.T.astype(BF16_NP))  # [e, o]
    b3bc = np.ascontiguousarray(
        np.broadcast_to(np.asarray(b_enc2in, f32), (P, D)))

    in_maps = []
    for c in range(N_CORES):
        bs = slice(c * BPC, (c + 1) * BPC)
        x = np.asarray(in_seq[:, bs, :], f32)
        t = np.asarray(prev_target_seq[:, bs, :], f32)
        e = np.asarray(enc_seq[:, bs, :], f32)
        # host glue: esum/v1/v2/c0 in fp64 (O(LBD + D^2) input/weight
        # reductions -- the ssum = X@v1 + T@v2 + c0 reformulation)
        esum = e.astype(f64).sum(axis=0)                    # [BPC, D]
        v1 = (esum @ W1.astype(f64)).astype(f32)            # [BPC, D]
        v2 = (esum @ W2.astype(f64)).astype(f32)
        c0 = (esum @ (np.asarray(b_in2enc, f64) + np.asarray(b_lab2enc, f64))
              ).astype(f32)                                  # [BPC]
        vecs = np.zeros((P, 22), f32)
        for b in range(BPC):
            vecs[:, V1C + b * NE : V1C + (b + 1) * NE] = v1[b].reshape(NE, P).T
            vecs[:, V2C + b * NE : V2C + (b + 1) * NE] = v2[b].reshape(NE, P).T
            vecs[:, C0C + b] = c0[b]
        vecs[:, BDC : BDC + NE] = bd.reshape(NE, P).T
        in_maps.append({
            "xT": np.ascontiguousarray(x.transpose(1, 2, 0)),
            "tT": np.ascontiguousarray(t.transpose(1, 2, 0)),
            "enb": np.ascontiguousarray(e.transpose(1, 0, 2).astype(BF16_NP)),
            "w1t": w1t, "w2t": w2t, "w3tb": w3tb,
            "vecs": vecs, "b3bc": b3bc,
        })
    return in_maps


_NC_CACHE = {}


def _get_nc():
    if "nc" not in _NC_CACHE:
        _NC_CACHE["nc"] = build_nc()
    return _NC_CACHE["nc"]


def kernel(**inputs):
    in_maps = _make_in_maps(**inputs)
    nc = _get_nc()
    res = run_bass_kernel_spmd(nc, in_maps, core_ids=list(range(N_CORES)))
    out = np.empty((L, B, D), np.float32)
    for c in range(N_CORES):
        per_core = res.results[c]["out"]  # (BPC, L, D)
        for j in range(BPC):
            out[:, c * BPC + j, :] = per_core[j]
    return out


def kernel_sim(core_id=0, **inputs):
    """CoreSim validation path: simulate one core, return its (BPC, L, D) output."""
    from concourse.bass_interp import CoreSim

    in_maps = _make_in_maps(**inputs)
    nc = _get_nc()
    sim = CoreSim(nc)
    for name, val in in_maps[core_id].items():
        sim.tensor(name)[:] = val
    sim.simulate(check_with_hw=False)
    return np.array(sim.tensor("out"))
